# revision 9
# baseline (speedup 1.0000x reference)
"""Trainium2 Bass kernel for nn_CustomLlamaAttention (partial-RoPE GQA attention
with low-rank KV, tensor-parallel over heads on 8 NeuronCores).

Self-contained: hardcodes shapes/sharding; builds one SPMD Bass program and runs
it on cores 0-7 via run_bass_kernel_spmd. Host pre-transposes/pre-casts inputs,
device computes everything transposed (feature-major), host re-assembles.

Sharding: core c owns q heads 4c..4c+3 (= KV head c), o_proj output-dim shard
c*256..(c+1)*256; hidden_states replicated.

v2: fully pipelined over 4 s-chunks of 512: per-chunk hsT load -> fused
projection -> rope/up-proj -> causal attention -> chunked AllGather -> o-proj.
Softmax denominators via DVE reciprocal_approx_fast (replacing the 3.3us
iterative divide); exp batched over head pairs ([128,1024] activations).
"""

import sys

for _p in ("/opt/trn_rl_repo",):
    if _p not in sys.path:
        sys.path.append(_p)

import numpy as np
import ml_dtypes

import concourse.bass as bass
import concourse.tile as tile
from concourse import bacc
from concourse import mybir
from concourse.bass import ts
from concourse.bass_utils import run_bass_kernel_spmd

# ---- problem constants (hardcoded per spec) ----
HID = 2048
NH = 32
NKV = 8
HD = 64
LR = 32
TOPK = 16
THETA = 10000.0
B, S = 1, 2048
NCORES = 8
HPC = NH // NCORES          # 4 q heads per core
QSH = HPC * HD              # 256 q rows per core
KRSH = 2 * TOPK             # 32 roped dims per KV head
NOPESH = HD - KRSH          # 32 nope dims per KV head
LAT = LR * NKV              # 256 latent
WCAT = LAT + QSH + KRSH     # 544 = [down 256 | q 256 | kr 32]
P = 128
NSC = S // 512              # 4 s-chunks of 512
NKT = S // P                # 16 k-tiles of 128
BF = mybir.dt.bfloat16
F32 = mybir.dt.float32

# m-chunks of wcat columns: [down0, down1, kr, q01, q23]
# (K-path chunks first so KT/V derivation can start before q rope)
MCHUNKS = [(0, 128), (128, 128), (512, 32), (256, 128), (384, 128)]


def _build_program():
    nc = bacc.Bacc(
        "TRN2",
        target_bir_lowering=False,
        debug=False,
        num_devices=NCORES,
    )

    # DRAM I/O (per-core data supplied via in_maps)
    hsT_d = nc.dram_tensor("hsT", [HID, S], BF, kind="ExternalInput").ap()
    wcatT_d = nc.dram_tensor("wcatT", [HID, WCAT], BF, kind="ExternalInput").ap()
    upkT_d = nc.dram_tensor("upkT", [LAT, NOPESH], BF, kind="ExternalInput").ap()
    upvT_d = nc.dram_tensor("upvT", [LAT, HD], BF, kind="ExternalInput").ap()
    owT_d = nc.dram_tensor("owT", [HID, QSH], BF, kind="ExternalInput").ap()
    cosq_d = nc.dram_tensor("cosq", [P, S], BF, kind="ExternalInput").ap()
    sinq_d = nc.dram_tensor("sinq", [P, S], BF, kind="ExternalInput").ap()
    cosk_d = nc.dram_tensor("cosk", [KRSH, S], BF, kind="ExternalInput").ap()
    sink_d = nc.dram_tensor("sink", [KRSH, S], BF, kind="ExternalInput").ap()
    mask_d = nc.dram_tensor("maskdiag", [P, 4, 1024], BF, kind="ExternalInput").ap()

    outT_d = nc.dram_tensor("outT", [QSH, S], F32, kind="ExternalOutput").ap()

    # internal DRAM for the per-chunk attention-output AllGathers
    og_in = [nc.dram_tensor(f"og_in{j}", [QSH, 512], BF).ap() for j in range(NSC)]
    og_out = [
        nc.dram_tensor(f"og_out{j}", [NH * HD, 512], BF, addr_space="Shared").ap()
        for j in range(NSC)
    ]

    Exp = mybir.ActivationFunctionType.Exp

    with tile.TileContext(nc) as tc:
        with (
            tc.tile_pool(name="sing", bufs=1) as sing,
            tc.tile_pool(name="hsp", bufs=2) as hsp,
            tc.tile_pool(name="otp", bufs=2) as otp,
            tc.tile_pool(name="tmp", bufs=3) as tmp,
            tc.tile_pool(name="apool", bufs=3) as apool,
            tc.tile_pool(name="npool", bufs=4) as npool,
            tc.tile_pool(name="psum_mm", bufs=2, space="PSUM") as psum_mm,
            tc.tile_pool(name="psum_s", bufs=2, space="PSUM") as psum_s,
            tc.tile_pool(name="psum_av", bufs=2, space="PSUM") as psum_av,
        ):
            # ---- persistent SBUF tiles ----
            wcat_sb = sing.tile([P, NKT, WCAT], BF, tag="wcat")
            ow_sb = sing.tile([P, NKT, QSH], BF, tag="ow")
            upk_sb = sing.tile([P, 2, NOPESH], BF, tag="upk")
            upv_sb = sing.tile([P, 2, HD], BF, tag="upv")
            cosq_sb = sing.tile([P, S], BF, tag="cosq")
            sinq_sb = sing.tile([P, S], BF, tag="sinq")
            cosk_sb = sing.tile([KRSH, S], BF, tag="cosk")
            sink_sb = sing.tile([KRSH, S], BF, tag="sink")
            mask_sb = sing.tile([P, 4, 1024], BF, tag="mask")
            yT = sing.tile([P, 5, S], BF, tag="yT")
            kcT = sing.tile([NOPESH, NKT, P], BF, tag="kcT")
            KT = sing.tile([P, NKT, P], BF, tag="KT")   # rows: 2 dup bands of 64 d
            V = sing.tile([P, NKT, HD + 1], BF, tag="V")  # col HD = ones
            qr0 = sing.tile([P, S], BF, tag="qr0")      # heads 0,1 (rope'd)
            qr1 = sing.tile([P, S], BF, tag="qr1")      # heads 2,3

            # ---- global loads ----
            # sync queue carries only AG-independent transfers (so chunk j+1's
            # hsT load is never stuck behind a collective-dependent DMA);
            # tables go out on the scalar HWDGE queue in parallel.
            nc.sync.dma_start(
                out=wcat_sb, in_=wcatT_d.rearrange("(ko p) m -> p ko m", p=P)
            )
            nc.scalar.dma_start(
                out=upk_sb, in_=upkT_d.rearrange("(ko p) m -> p ko m", p=P)
            )
            nc.scalar.dma_start(
                out=upv_sb, in_=upvT_d.rearrange("(ko p) m -> p ko m", p=P)
            )
            nc.scalar.dma_start(out=cosq_sb, in_=cosq_d)
            nc.scalar.dma_start(out=sinq_sb, in_=sinq_d)
            nc.scalar.dma_start(out=cosk_sb, in_=cosk_d)
            nc.scalar.dma_start(out=sink_sb, in_=sink_d)
            nc.scalar.dma_start(out=mask_sb, in_=mask_d)
            nc.scalar.dma_start(
                out=ow_sb, in_=owT_d.rearrange("(ko p) m -> p ko m", p=P)
            )

            nc.vector.memset(V[:, :, HD : HD + 1], 1.0)

            for scj in range(NSC):
                sl = ts(scj, 512)
                kts = ts(scj, 4)

                # -- per-chunk hsT load --
                h = hsp.tile([P, NKT, 512], BF, tag="hs")
                nc.sync.dma_start(
                    out=h, in_=hsT_d.rearrange("(ko p) s -> p ko s", p=P)[:, :, sl]
                )

                # -- phase 1: fused projection for this chunk --
                for m0, msz in MCHUNKS:
                    ps = psum_mm.tile([P, 512], F32, tag="mm")
                    for kt in range(NKT):
                        nc.tensor.matmul(
                            ps[:msz],
                            lhsT=wcat_sb[:, kt, m0 : m0 + msz],
                            rhs=h[:, kt, :],
                            start=(kt == 0),
                            stop=(kt == NKT - 1),
                        )
                    mi = 4 if m0 == 512 else m0 // 128
                    nc.any.tensor_copy(out=yT[:msz, mi, sl], in_=ps[:msz])

                # -- up-proj K (nope rows) for this chunk --
                ps = psum_mm.tile([P, 512], F32, tag="mm")
                for lt in range(2):
                    nc.tensor.matmul(
                        ps[:NOPESH],
                        lhsT=upk_sb[:, lt, :],
                        rhs=yT[:, lt, sl],
                        start=(lt == 0),
                        stop=(lt == 1),
                    )
                nc.any.tensor_copy(
                    out=kcT[:, kts, :],
                    in_=ps[:NOPESH].rearrange("p (ko ki) -> p ko ki", ki=P),
                )

                # -- up-proj V for this chunk's 4 k-tiles --
                for m in range(4 * scj, 4 * scj + 4):
                    ps = psum_mm.tile([P, 512], F32, tag="mm")
                    for lt in range(2):
                        nc.tensor.matmul(
                            ps[:, 0:HD],
                            lhsT=yT[:, lt, ts(m, P)],
                            rhs=upv_sb[:, lt, :],
                            start=(lt == 0),
                            stop=(lt == 1),
                        )
                    nc.any.tensor_copy(out=V[:, m, 0:HD], in_=ps[:, 0:HD])

                # -- kr rope for this chunk (rope pairs are (r, r+16)) --
                krT = yT[0:KRSH, 4, sl]
                ksh = tmp.tile([KRSH, 512], BF, tag="ksh")
                nc.sync.dma_start(out=ksh[0:16], in_=krT[16:32])
                nc.sync.dma_start(out=ksh[16:32], in_=krT[0:16])
                krot = tmp.tile([KRSH, 512], BF, tag="krot")
                nc.vector.tensor_mul(out=krot, in0=krT, in1=cosk_sb[:, sl])
                nc.vector.tensor_mul(out=ksh, in0=ksh, in1=sink_sb[:, sl])
                nc.vector.tensor_add(out=krot, in0=krot, in1=ksh)

                # -- scatter rope'd + nope K rows into KT bands (dup 2x) --
                for b in (0, 64):
                    nc.sync.dma_start(
                        out=KT[b + 0 : b + 16, kts, :],
                        in_=krot[0:16].rearrange("p (ko ki) -> p ko ki", ki=P),
                    )
                    nc.sync.dma_start(
                        out=KT[b + 32 : b + 48, kts, :],
                        in_=krot[16:32].rearrange("p (ko ki) -> p ko ki", ki=P),
                    )
                    nc.sync.dma_start(
                        out=KT[b + 16 : b + 32, kts, :], in_=kcT[0:16, kts, :]
                    )
                    nc.sync.dma_start(
                        out=KT[b + 48 : b + 64, kts, :], in_=kcT[16:32, kts, :]
                    )

                # -- q rope for this chunk (scale 1/sqrt(HD) folded into tables) --
                for g, dst in ((2, qr0), (3, qr1)):
                    qt = yT[:, g, sl]
                    qsh = tmp.tile([P, 512], BF, tag="qsh")
                    for b in (0, 64):
                        nc.vector.tensor_copy(out=qsh[b : b + 32], in_=qt[b + 32 : b + 64])
                        nc.vector.tensor_copy(out=qsh[b + 32 : b + 64], in_=qt[b : b + 32])
                    nc.vector.tensor_mul(out=dst[:, sl], in0=qt, in1=cosq_sb[:, sl])
                    nc.vector.tensor_mul(out=qsh, in0=qsh, in1=sinq_sb[:, sl])
                    nc.vector.tensor_add(out=dst[:, sl], in0=dst[:, sl], in1=qsh)

                # -- causal attention for this q-chunk, 4 local heads --
                j = scj
                nkt = 4 * j + 4
                for p_i, qr in enumerate((qr0, qr1)):
                    av = [
                        psum_av.tile([HD + 1, 512], F32, tag="av", name=f"av0_{j}_{p_i}"),
                        psum_av.tile([HD + 1, 512], F32, tag="av", name=f"av1_{j}_{p_i}"),
                    ]
                    for kt in range(nkt):
                        ss = psum_s.tile([P, 1024], F32, tag="s", name=f"s_{j}_{p_i}_{kt}")
                        for hb, b0 in ((0, 0), (1, 64)):
                            nc.tensor.matmul(
                                ss[:, ts(hb, 512)],
                                lhsT=KT[b0 : b0 + 64, kt, :],
                                rhs=qr[b0 : b0 + 64, sl],
                                start=True,
                                stop=True,
                            )
                        a = apool.tile([P, 1024], BF, tag="a")
                        nc.scalar.activation(a, ss, Exp)
                        if kt >= 4 * j:
                            nc.vector.tensor_mul(
                                out=a, in0=a, in1=mask_sb[:, kt - 4 * j, :]
                            )
                        for hb in (0, 1):
                            nc.tensor.matmul(
                                av[hb],
                                lhsT=V[:, kt, :],
                                rhs=a[:, ts(hb, 512)],
                                start=(kt == 0),
                                stop=(kt == nkt - 1),
                            )
                    # normalize: 1/denominator via fast DVE reciprocal
                    # (denominator row staged to SBUF partition 0 first — the
                    # custom-DVE reciprocal needs a base-aligned SBUF input)
                    for hb in (0, 1):
                        h_loc = 2 * p_i + hb
                        den = npool.tile([1, 512], F32, tag="den")
                        nc.vector.tensor_copy(out=den, in_=av[hb][HD : HD + 1])
                        rc = npool.tile([1, 512], F32, tag="rc")
                        nc.vector.reciprocal_approx_fast(out=rc, in_=den)
                        bc = npool.tile([HD, 512], F32, tag="bc")
                        nc.gpsimd.partition_broadcast(bc, rc, channels=HD)
                        on = npool.tile([HD, 512], BF, tag="on")
                        nc.vector.tensor_mul(out=on, in0=av[hb][0:HD], in1=bc)
                        nc.gpsimd.dma_start(
                            out=og_in[j][h_loc * HD : (h_loc + 1) * HD, :], in_=on
                        )

                # -- chunked AllGather of attention outputs --
                nc.gpsimd.collective_compute(
                    "AllGather",
                    mybir.AluOpType.bypass,
                    replica_groups=[list(range(NCORES))],
                    ins=[og_in[j]],
                    outs=[og_out[j]],
                )

                # -- o-projection for this chunk --
                # ot load + outT store ride the gpsimd queue: they depend on
                # the collective, and a stalled gpsimd queue doesn't gate the
                # next chunk's projection work.
                ot = otp.tile([P, NKT, 512], BF, tag="ot")
                nc.gpsimd.dma_start(
                    out=ot, in_=og_out[j].rearrange("(ko p) s -> p ko s", p=P)
                )
                for mc in range(QSH // P):
                    ps = psum_mm.tile([P, 512], F32, tag="mm")
                    for kt in range(NKT):
                        nc.tensor.matmul(
                            ps,
                            lhsT=ow_sb[:, kt, ts(mc, P)],
                            rhs=ot[:, kt, :],
                            start=(kt == 0),
                            stop=(kt == NKT - 1),
                        )
                    o_sb = tmp.tile([P, 512], F32, tag="out")
                    nc.any.tensor_copy(out=o_sb, in_=ps)
                    nc.gpsimd.dma_start(out=outT_d[ts(mc, P), sl], in_=o_sb)

    nc.compile()
    return nc


_NC_CACHE = None


def _get_program():
    global _NC_CACHE
    if _NC_CACHE is None:
        _NC_CACHE = _build_program()
    return _NC_CACHE


def _bf16(x):
    return np.asarray(x, dtype=np.float32).astype(ml_dtypes.bfloat16)


def _host_inputs(hidden_states, q_w, kr_w, down_w, upk_w, upv_w, o_w):
    hs = np.asarray(hidden_states, dtype=np.float32)[0]  # [S, HID]
    q_w = np.asarray(q_w, np.float32)
    kr_w = np.asarray(kr_w, np.float32)
    down_w = np.asarray(down_w, np.float32)
    upk_w = np.asarray(upk_w, np.float32)
    upv_w = np.asarray(upv_w, np.float32)
    o_w = np.asarray(o_w, np.float32)

    hsT = _bf16(hs.T)  # [HID, S]

    # RoPE tables (fp32 host math, bf16 on device)
    pos = np.arange(S, dtype=np.float32)
    inv = 1.0 / (THETA ** (np.arange(0, HD, 2, dtype=np.float32) / HD))
    fr = pos[:, None] * inv[None, :]           # [S, 32]
    emb = np.concatenate([fr, fr], -1)         # [S, 64]
    cosT = np.cos(emb).T                       # [64, S]
    sinT = np.sin(emb).T
    sc = 1.0 / np.sqrt(np.float32(HD))

    cosq = np.tile(cosT, (2, 1)) * sc          # [128, S]
    sgn = np.where(np.arange(HD) < 32, -1.0, 1.0).astype(np.float32)[:, None]
    sinq = np.tile(sinT * sgn, (2, 1)) * sc    # [128, S]

    rope_d = np.concatenate([np.arange(0, 16), np.arange(32, 48)])
    cosk = cosT[rope_d]                        # [32, S]
    sgnk = np.where(np.arange(KRSH) < 16, -1.0, 1.0).astype(np.float32)[:, None]
    sink = sinT[rope_d] * sgnk

    # diagonal causal masks for the 4 k-tile offsets within a 512 q-chunk,
    # duplicated along the free dim for the two packed heads
    kk = np.arange(P)[:, None]
    qq = np.arange(512)[None, :]
    mask = np.stack(
        [(P * i + kk <= qq).astype(np.float32) for i in range(4)], axis=1
    )  # [128, 4, 512]
    mask2 = np.concatenate([mask, mask], axis=-1)  # [128, 4, 1024]

    shared = {
        "hsT": hsT,
        "cosq": _bf16(cosq),
        "sinq": _bf16(sinq),
        "cosk": _bf16(cosk),
        "sink": _bf16(sink),
        "maskdiag": _bf16(mask2),
    }
    in_maps = []
    for c in range(NCORES):
        q_rows = q_w[c * QSH : (c + 1) * QSH]          # [256, HID]
        kr_rows = kr_w[c * KRSH : (c + 1) * KRSH]      # [32, HID]
        wcat = np.concatenate([down_w, q_rows, kr_rows], axis=0)  # [544, HID]
        m = dict(shared)
        m["wcatT"] = _bf16(wcat.T)                     # [HID, 544]
        m["upkT"] = _bf16(upk_w[c * NOPESH : (c + 1) * NOPESH].T)  # [256, 32]
        m["upvT"] = _bf16(upv_w[c * HD : (c + 1) * HD].T)          # [256, 64]
        m["owT"] = _bf16(o_w[c * QSH : (c + 1) * QSH].T)           # [HID, 256]
        in_maps.append(m)
    return in_maps


def kernel(**inputs) -> np.ndarray:
    nc = _get_program()
    in_maps = _host_inputs(**inputs)
    res = run_bass_kernel_spmd(nc, in_maps, core_ids=list(range(NCORES)))
    outT = np.concatenate(
        [np.asarray(res.results[c]["outT"]) for c in range(NCORES)], axis=0
    )  # [2048, S]
    return np.ascontiguousarray(outT.T)[None].astype(np.float32)


if __name__ == "__main__":
    rng = np.random.default_rng(0)
    ins = {
        "hidden_states": rng.standard_normal((B, S, HID), dtype=np.float32),
        "q_w": rng.standard_normal((NH * HD, HID), dtype=np.float32) * 0.02,
        "kr_w": rng.standard_normal((2 * TOPK * NKV, HID), dtype=np.float32) * 0.02,
        "down_w": rng.standard_normal((LAT, HID), dtype=np.float32) * 0.02,
        "upk_w": rng.standard_normal((NOPESH * NKV, LAT), dtype=np.float32) * 0.02,
        "upv_w": rng.standard_normal((NKV * HD, LAT), dtype=np.float32) * 0.02,
        "o_w": rng.standard_normal((HID, NH * HD), dtype=np.float32) * 0.02,
    }
    out = kernel(**ins)
    print(out.shape, out.dtype, float(np.abs(out).max()))
